# revision 15
# baseline (speedup 1.0000x reference)
"""Soft-kNN imputation kernel for Trainium2 (8 NeuronCores, SPMD).

Problem: for a single query X_missing [64], over X_train [1M, 64]:
  d_i   = ||x_i - q||_2
  w_i   = softmax(-d_i)            (tau = 1.0)
  out   = sum over top-32 w_i * y_train[i]     -> [1, 64]

Sharding: X_train is split along N across the 8 cores (125,000 rows
each). y_train never touches the device - only 32 of its rows are ever
needed, and the host gathers them at the end.

Per-core pipeline (memory-bound: streams the 32 MB shard exactly once;
the SDMA engines sustain ~425 GB/s when nothing perturbs the issue
chain):

  The host pre-transposes the shard into a feature-major "2-block"
  layout (two train rows per column, features stacked on partitions
  0-63 / 64-127). Per supertile of up to 32 chunks (a chunk = 128
  columns = 256 rows):

    DMA  (HWDGE)  f32 supertile  ->  SBUF xs
    ACT  Square(x - q) with per-partition bias=-q, output *bf16*
    PE   one [128,128]-stationary x [128,2]-selector matmul per chunk;
         the selector is -1, so *negated* d^2 lands as 2 f32 columns
         per chunk in a persistent PSUM accumulator (one 512-column
         bank per range, no drain, PE streams back-to-back).

  bf16 squares make LDWEIGHTS ~4x cheaper than the f32r alternative
  (FWL engages for non-fp32 128-column weights), which is what keeps
  the PE far under the DMA roofline; the bf16 rounding of the 64
  summed squares perturbs d by ~1e-3 absolute - far inside tolerance.

  CRITICAL scheduling invariant: DMA issue order is gated on the ACT
  instruction-completion count (Tile recycles the stream buffers by
  counting ACT completions), so ANY extra ACT instruction mid-stream
  (an activation-table load, a sqrt/exp) delays every later DMA issue
  and stretches the stream. ACT therefore runs NOTHING but the 19
  squares until the last matmul. The per-range top-8 candidate
  extraction runs entirely on the otherwise-idle DVE (copy the
  finished -d^2 PSUM bank to SBUF, max8, max_index - PE is writing a
  different bank, so no PSUM bank conflicts), with ranges A and B
  mid-stream and the small range C in the tail. The single tail
  sqrt(-(-d^2)) -> exp(-d) pass over all 978 columns exists only to
  produce the softmax denominator (accum_out); candidate weights are
  recomputed exactly on the host from the shipped -d^2 values.

  All outputs live in ONE packed u32 tile (-d^2 bitcast, indices,
  denominator) written by a single DMA - separate small outputs cost
  multi-us completion waits.

The host merges the 8 x 128 x (3x8) candidates (any global top-32
element is necessarily in its own partition-range's top-8), finishes
the softmax normalization, and does the 32-row gather from y_train plus
the tiny weighted [32, 64] reduction.
"""

import numpy as np

N = 1_000_000
D = 64
K = 32
NCORES = 8
SHARD = N // NCORES            # 125000 rows per core
PROWS = 128                    # SBUF partitions

CHUNK_ROWS = 256               # rows per PE chunk (2 blocks x 128)
NCHUNK = 489                   # chunks per core (125184 rows, padded)
PAD_ROWS = NCHUNK * CHUNK_ROWS # 125184
# Supertile schedule: ramp up (prime the pipeline), 32-chunk steady
# state (2 MB DMAs), short ramp down (tiny last-supertile latency).
ST_SIZES = [4, 8, 16, 30] + [48] * 8 + [16, 12, 8, 6, 3, 2]
assert sum(ST_SIZES) == NCHUNK
MAX_ST = max(ST_SIZES)

# -d^2 accumulates in three bank-aligned PSUM ranges (one 2 KiB bank
# each); A and B are scanned mid-stream by DVE, C in the tail.
RANGES = [(0, 256), (256, 400), (400, NCHUNK)]   # chunks
PSCOLS = 512                   # one PSUM bank = 512 f32 columns
NCAND = 8 * len(RANGES)
PACKW = 2 * NCAND + 1          # packed output: -d^2, idx, z

PAD_VAL = 1.0e4                # sentinel: d ~ 1e4 -> exp(-d) == 0.0 in f32
_CACHE = {}
LAST_RESULTS = None            # BassKernelResults of the most recent run


def _build_nc():
    import concourse.bacc as bacc
    import concourse.tile as tile
    from concourse import mybir

    f32 = mybir.dt.float32
    bf16 = mybir.dt.bfloat16
    u32 = mybir.dt.uint32

    # Bacc (not plain Bass): its compile() pipeline runs
    # generate_event_semaphores, which splits multi-semaphore waits into
    # event-semaphore chains - the TRN2 ISA allows at most one wait per
    # instruction and walrus rejects unsplit programs.
    nc = bacc.Bacc("TRN2", target_bir_lowering=False, debug=False)
    xt2_d = nc.dram_tensor(
        "xt2", [PROWS, NCHUNK * PROWS], f32, kind="ExternalInput"
    ).ap()
    nq_d = nc.dram_tensor("negq", [PROWS, 1], f32, kind="ExternalInput").ap()
    # -1/0 selector: exact in bf16.
    sel_d = nc.dram_tensor("sel", [PROWS, 2], bf16, kind="ExternalInput").ap()
    out_d = nc.dram_tensor(
        "packed", [PROWS, PACKW], u32, kind="ExternalOutput"
    ).ap()

    with tile.TileContext(nc) as tc:
        with (
            tc.tile_pool(name="persist", bufs=1) as persist,
            tc.tile_pool(name="xs", bufs=4) as xs_pool,
            tc.tile_pool(name="sq", bufs=4) as sq_pool,
            tc.tile_pool(name="psum", bufs=1, space="PSUM") as psum_pool,
        ):
            negq = persist.tile([PROWS, 1], f32)
            sel = persist.tile([PROWS, 2], bf16)
            nd2 = persist.tile([PROWS, 2 * NCHUNK], f32)   # -d^2 (SBUF)
            d2 = persist.tile([PROWS, 2 * NCHUNK], f32)    # +d, then unused
            wt = persist.tile([PROWS, 2 * NCHUNK], f32)    # exp(-d) scratch
            pack = persist.tile([PROWS, PACKW], u32)

            ps = [
                psum_pool.tile([PROWS, PSCOLS], f32, name=f"ps{r}")
                for r in range(len(RANGES))
            ]

            # First data supertile goes out before the tiny helper DMAs:
            # it is the stream-critical one.
            xs0 = xs_pool.tile([PROWS, MAX_ST * PROWS], f32, tag="xs")
            nc.sync.dma_start(
                out=xs0[:, : ST_SIZES[0] * PROWS],
                in_=xt2_d[:, : ST_SIZES[0] * PROWS],
            )
            nc.sync.dma_start(out=negq[:], in_=nq_d[:])
            nc.sync.dma_start(out=sel[:], in_=sel_d[:])

            def scan(r):
                """DVE-only: -d^2 PSUM bank -> SBUF, then top-8."""
                c0, c1 = RANGES[r]
                nv = nd2[:, 2 * c0 : 2 * c1]
                nc.vector.tensor_scalar(
                    nv,
                    ps[r][:, : 2 * (c1 - c0)],
                    0.0,
                    scalar2=None,
                    op0=mybir.AluOpType.add,
                )
                v8 = pack[:, r * 8 : (r + 1) * 8].bitcast(f32)
                i8 = pack[:, NCAND + r * 8 : NCAND + (r + 1) * 8]
                nc.vector.max(out=v8, in_=nv)
                nc.vector.max_index(out=i8, in_max=v8, in_values=nv)

            done = 0
            scanned = 0
            for sti, g in enumerate(ST_SIZES):
                fd = g * PROWS
                if sti == 0:
                    xs = xs0
                else:
                    xs = xs_pool.tile([PROWS, MAX_ST * PROWS], f32, tag="xs")
                    nc.sync.dma_start(
                        out=xs[:, :fd],
                        in_=xt2_d[:, done * PROWS : done * PROWS + fd],
                    )
                sq = sq_pool.tile([PROWS, MAX_ST * PROWS], bf16, tag="sq")
                nc.scalar.activation(
                    sq[:, :fd],
                    xs[:, :fd],
                    mybir.ActivationFunctionType.Square,
                    bias=negq[:],
                )
                for j in range(g):
                    c = done + j
                    r = next(
                        i for i, (c0, c1) in enumerate(RANGES) if c0 <= c < c1
                    )
                    cc = 2 * (c - RANGES[r][0])
                    nc.tensor.matmul(
                        out=ps[r][:, cc : cc + 2],
                        lhsT=sq[:, j * PROWS : (j + 1) * PROWS],
                        rhs=sel[:],
                        start=True,
                        stop=True,
                    )
                done += g
                while scanned < len(RANGES) - 1 and done >= RANGES[scanned][1]:
                    scan(scanned)
                    scanned += 1

            scan(len(RANGES) - 1)

            # d = sqrt(-(-d^2)); zp = sum_j exp(-d_j) (tail-only ACT work;
            # the two activation-table loads land here, after the last
            # square, where they gate nothing).
            nc.scalar.activation(
                d2[:],
                nd2[:],
                mybir.ActivationFunctionType.Sqrt,
                scale=-1.0,
            )
            nc.scalar.activation(
                wt[:],
                d2[:],
                mybir.ActivationFunctionType.Exp,
                scale=-1.0,
                accum_out=pack[:, PACKW - 1 : PACKW].bitcast(f32),
            )

            nc.sync.dma_start(out=out_d[:], in_=pack[:])

    nc.compile()
    return nc


def _pe_layout(xc):
    """[PAD_ROWS, D] rows -> feature-major 2-block layout [128, NCHUNK*128].

    xt2[b*64+k, j*128+m] = xc[j*256 + b*128 + m, k]
    """
    r = xc.reshape(NCHUNK, 2, PROWS, D)          # [j, b, m, k]
    return np.ascontiguousarray(
        r.transpose(1, 3, 0, 2).reshape(PROWS, NCHUNK * PROWS)
    )


def kernel(X_train, y_train, X_missing):
    import os

    import ml_dtypes
    from concourse.bass_utils import run_bass_kernel_spmd

    global LAST_RESULTS

    X_train = np.ascontiguousarray(np.asarray(X_train, dtype=np.float32))
    y_train = np.asarray(y_train, dtype=np.float32)
    X_missing = np.asarray(X_missing, dtype=np.float32)

    if "nc" not in _CACHE:
        _CACHE["nc"] = _build_nc()
    nc = _CACHE["nc"]

    negq = np.ascontiguousarray(
        -np.concatenate([X_missing, X_missing])[:, None]
    )  # [128, 1]
    sel = np.zeros((PROWS, 2), np.float32)
    sel[:D, 0] = -1.0
    sel[D:, 1] = -1.0
    sel = sel.astype(ml_dtypes.bfloat16)

    in_maps = []
    for c in range(NCORES):
        xc = np.full((PAD_ROWS, D), PAD_VAL, dtype=np.float32)
        xc[:SHARD] = X_train[c * SHARD : (c + 1) * SHARD]
        in_maps.append({"xt2": _pe_layout(xc), "negq": negq, "sel": sel})

    trace = bool(int(os.environ.get("KNN_TRACE", "0")))
    res = run_bass_kernel_spmd(
        nc, in_maps, core_ids=list(range(NCORES)), trace=trace
    )
    LAST_RESULTS = res

    # Host-side merge: global softmax denominator + global top-32 among
    # the per-partition-range top-8 candidates, then the 32-row gather.
    base = np.repeat([2 * c0 for c0, _ in RANGES], 8)[None, :]  # [1, 24]
    p = np.arange(PROWS, dtype=np.int64)[:, None]
    z_total = 0.0
    all_vals = []
    all_rows = []
    for c in range(NCORES):
        packed = res.results[c]["packed"]
        v = (
            np.ascontiguousarray(packed[:, :NCAND])
            .view(np.float32)
            .reshape(-1)
        )  # -d^2
        jcol = packed[:, NCAND : 2 * NCAND].astype(np.int64) + base
        z_total += float(
            np.ascontiguousarray(packed[:, 2 * NCAND :])
            .view(np.float32)
            .astype(np.float64)
            .sum()
        )
        local_row = (jcol // 2) * CHUNK_ROWS + (jcol % 2) * PROWS + p
        rows = (c * SHARD + local_row).reshape(-1)
        keep = local_row.reshape(-1) < SHARD
        all_vals.append(v[keep])
        all_rows.append(rows[keep])
    all_vals = np.concatenate(all_vals)
    all_rows = np.concatenate(all_rows)

    sel_i = np.argpartition(-all_vals, K - 1)[:K]           # largest -d^2
    w = np.exp(-np.sqrt(np.maximum(-all_vals[sel_i], 0.0).astype(np.float64)))
    w /= z_total
    out = (w[:, None] * y_train[all_rows[sel_i]].astype(np.float64)).sum(axis=0)
    return out[None, :].astype(np.float32)
